# revision 1
# baseline (speedup 1.0000x reference)
"""Trainium2 Bass kernel for nn_CustomLoss_74826920231413.

Loss structure (B=32, E=1024, K=20):
    c  = complex(nnOutput[:, :NOUT], nnOutput[:, NOUT:])
    d  = c[:, :K];  U = c[:, K:VLOC].reshape(B,E,K);  V = c[:, VLOC:].reshape(B,E,K)
    obj1/obj2 = sum_{j<k} |U^T U| / B (no conj), same for V
    pred = U @ diag(d) @ V^T;  tk = complex(kern_real, kern_imag)
    loss = ||tk - pred||^2 / ||tk||^2 + 0.01*(obj1+obj2)

Device strategy (data-parallel over B, 4 batch rows per core, 8 cores):
    ||tk - pred||^2 = ||tk||^2 - 2*Re<conj(tk),pred> + ||pred||^2, so the
    device only needs one streaming pass over tk producing small outputs:
      * gram[b]  = [Ur|Ui]^T[Ur|Ui] and [Vr|Vi]^T[Vr|Vi]  -> objs, ||pred||^2
      * yr[b]    = W^T tkr with W = [Ur|Ui]      (40x1024) -> cross term
      * yi[b]    = W^T tki                        (40x1024)
      * den partials = per-partition sums of tk^2
    Host assembles the three scalars from these partials in float64.

    tk is shipped to the device as fp16: the loss is a ratio of O(1e9)
    quantities and 16-bit rounding of tk perturbs it at ~1e-6 relative
    (validated numerically), while halving the dominant DMA traffic.
    Gram runs in exact fp32 from the fp32 nnOutput. All input streams ride
    the sync HWDGE ring in host-prepacked partition-major layout (16KB
    contiguous lines); output stores ride gpsimd SWDGE queues.
"""

import sys

for _p in ("/opt/trn_rl_repo", "/root/.axon_site/_ro/trn_rl_repo"):
    if _p not in sys.path:
        sys.path.append(_p)

import numpy as np

import concourse.bacc as bacc
import concourse.mybir as mybir
import concourse.tile as tile
from concourse.bass_utils import run_bass_kernel_spmd

# Problem constants (hardcoded per harness contract)
E = 1024
K = 20
NOUT = K * (2 * E + 1)          # 40980
VLOC = K + K * E                # 20500
PENALTY = 0.01
B = 32
NCORES = 8
NB = B // NCORES                # batch rows per core
NCH = E // 128                  # 8 e-chunks of 128 partitions
HALF = NCH // 2                 # tk DMA split granularity (chunks per DMA)
F32 = mybir.dt.float32
F16 = mybir.dt.float16

_PROGRAM_CACHE = {}


def _build_program():
    """Per-core SPMD Bass program. Same program on all 8 cores; each core
    receives its own 4-row slice of the inputs (host-packed layouts)."""
    nc = bacc.Bacc("TRN2", target_bir_lowering=False, debug=False)

    # host-packed [Ur|Ui|Vr|Vi] fp32, partition-major: [b, p, c, 80]
    xuv_d = nc.dram_tensor("xuv", [NB, 128, NCH, 80], F32, kind="ExternalInput").ap()
    # host-packed fp16 [Ur|Ui] weights: [b, p, c, 40]
    w_d = nc.dram_tensor("w16", [NB, 128, NCH, 40], F16, kind="ExternalInput").ap()
    # host-packed fp16 kernels, partition-major: [b, p, c, f], e = c*128+p.
    # 16KB contiguous per partition line -> few DMA descriptors, so a single
    # HWDGE ring feeds the full HBM bandwidth.
    tkr_d = nc.dram_tensor("tkr", [NB, 128, NCH, E], F16, kind="ExternalInput").ap()
    tki_d = nc.dram_tensor("tki", [NB, 128, NCH, E], F16, kind="ExternalInput").ap()

    gram_d = nc.dram_tensor("gram", [NB, 40, 80], F32, kind="ExternalOutput").ap()
    yr_d = nc.dram_tensor("yr", [NB, 40, E], F32, kind="ExternalOutput").ap()
    yi_d = nc.dram_tensor("yi", [NB, 40, E], F32, kind="ExternalOutput").ap()
    den_d = nc.dram_tensor(
        "den", [2, 128, NB * NCH * 2], F32, kind="ExternalOutput"
    ).ap()

    mult = mybir.AluOpType.mult
    Square = mybir.ActivationFunctionType.Square

    with tile.TileContext(nc) as tc:
        with (
            tc.tile_pool(name="xuv", bufs=2) as xpool,
            tc.tile_pool(name="tk", bufs=3) as tkpool,
            tc.tile_pool(name="scr", bufs=2) as scrpool,
            tc.tile_pool(name="evac", bufs=2) as evacpool,
            tc.tile_pool(name="den", bufs=1) as denpool,
            tc.tile_pool(name="psg", bufs=2, space="PSUM") as psg_pool,
            tc.tile_pool(name="psy", bufs=1, space="PSUM") as psy_pool,
        ):
            # den accumulator columns; each engine owns its own tile (no
            # cross-engine write conflicts). col = (b*NCH + c)*2 + mat
            den_dve = denpool.tile([128, NB * NCH * 2], F32, name="den_dve")
            den_act = denpool.tile([128, NB * NCH * 2], F32, name="den_act")
            nc.vector.memset(den_dve[:], 0.0)
            nc.vector.memset(den_act[:], 0.0)

            for b in range(NB):
                # ---- kernels, fp16, halves for pipelining: [p, c, f]
                tkr_sb = []
                tki_sb = []
                for h in range(NCH // HALF):
                    cs = slice(h * HALF, (h + 1) * HALF)
                    tr = tkpool.tile([128, HALF, E], F16, name=f"tkr_h{h}")
                    nc.sync.dma_start(tr[:], tkr_d[b, :, cs])
                    tkr_sb.append(tr)
                    ti = tkpool.tile([128, HALF, E], F16, name=f"tki_h{h}")
                    nc.sync.dma_start(ti[:], tki_d[b, :, cs])
                    tki_sb.append(ti)

                def tkr_c(c):
                    return tkr_sb[c // HALF][:, c % HALF, :]

                def tki_c(c):
                    return tki_sb[c // HALF][:, c % HALF, :]

                # ---- U/V tile (fp32) + fp16 Y weights, host-packed layouts
                x_sb = xpool.tile([128, NCH, 80], F32, name="x_sb")
                nc.sync.dma_start(x_sb[:], xuv_d[b])
                w_sb = xpool.tile([128, NCH, 40], F16, name="w_sb")
                nc.sync.dma_start(w_sb[:], w_d[b])

                # ---- Grams: S_U = [Ur|Ui]^T [Ur|Ui], S_V likewise (exact
                # fp32). The U-V cross blocks are never needed by the host.
                ps_g = psg_pool.tile([40, 80], F32, name="ps_g")
                for c in range(NCH):
                    xu = x_sb[:, c, 0:40]
                    nc.tensor.matmul(
                        ps_g[:, 0:40], xu, xu, start=(c == 0), stop=(c == NCH - 1)
                    )
                for c in range(NCH):
                    xv = x_sb[:, c, 40:80]
                    nc.tensor.matmul(
                        ps_g[:, 40:80], xv, xv, start=(c == 0), stop=(c == NCH - 1)
                    )
                g_sb = evacpool.tile([40, 80], F32, name="g_sb")
                nc.vector.tensor_copy(g_sb[:], ps_g[:])
                nc.gpsimd.dma_start(gram_d[b], g_sb[:])

                # ---- Y: yr[j,f] = sum_e W[e,j] tkr[e,f], W = [Ur|Ui] (fp16)
                ps_yr = psy_pool.tile([40, E], F32, name="ps_yr")
                ps_yi = psy_pool.tile([40, E], F32, name="ps_yi")
                for c in range(NCH):
                    w = w_sb[:, c, :]
                    for h in range(2):
                        fs = slice(h * 512, (h + 1) * 512)
                        nc.tensor.matmul(
                            ps_yr[:, fs],
                            w,
                            tkr_c(c)[:, fs],
                            start=(c == 0),
                            stop=(c == NCH - 1),
                        )
                        nc.tensor.matmul(
                            ps_yi[:, fs],
                            w,
                            tki_c(c)[:, fs],
                            start=(c == 0),
                            stop=(c == NCH - 1),
                        )
                yr_sb = evacpool.tile([40, E], F32, name="yr_sb")
                nc.scalar.copy(yr_sb[:], ps_yr[:])
                nc.gpsimd.dma_start(yr_d[b], yr_sb[:])
                yi_sb = evacpool.tile([40, E], F32, name="yi_sb")
                nc.scalar.copy(yi_sb[:], ps_yi[:])
                nc.gpsimd.dma_start(yi_d[b], yi_sb[:])

                # ---- den partials: sum of squares along free dim (fp32
                # accumulate). Alternate units between DVE and ACT so the
                # post-stream straggler work is split across both engines.
                for c in range(NCH):
                    for mat, src in ((0, tkr_c(c)), (1, tki_c(c))):
                        idx = c * 2 + mat
                        col = (b * NCH + c) * 2 + mat
                        if idx % 2 == 0:
                            scr_v = scrpool.tile([128, E], F16, name="scr_v")
                            nc.vector.scalar_tensor_tensor(
                                scr_v[:],
                                src,
                                1.0,
                                src,
                                mult,
                                mult,
                                accum_out=den_dve[:, col:col + 1],
                            )
                        else:
                            scr_a = scrpool.tile([128, E], F16, name="scr_a")
                            nc.scalar.activation(
                                scr_a[:],
                                src,
                                Square,
                                accum_out=den_act[:, col:col + 1],
                            )

            nc.gpsimd.dma_start(den_d[0], den_dve[:])
            nc.gpsimd.dma_start(den_d[1], den_act[:])

    nc.compile()
    return nc


def _get_program():
    if "nc" not in _PROGRAM_CACHE:
        _PROGRAM_CACHE["nc"] = _build_program()
    return _PROGRAM_CACHE["nc"]


def _pack_inputs(nn, tkr, tki):
    """Host-side packing: per-core input dicts with device-friendly layouts."""
    # partition-major fp16: [B, E, E] -> [B, p, c, f] with e = c*128 + p
    tkr16 = np.ascontiguousarray(
        tkr.astype(np.float16).reshape(B, NCH, 128, E).transpose(0, 2, 1, 3)
    )
    tki16 = np.ascontiguousarray(
        tki.astype(np.float16).reshape(B, NCH, 128, E).transpose(0, 2, 1, 3)
    )
    # [B, E, K] slices of nn
    Ur = nn[:, K:VLOC].reshape(B, E, K)
    Ui = nn[:, NOUT + K:NOUT + VLOC].reshape(B, E, K)
    Vr = nn[:, VLOC:NOUT].reshape(B, E, K)
    Vi = nn[:, NOUT + VLOC:2 * NOUT].reshape(B, E, K)
    xuv = np.concatenate([Ur, Ui, Vr, Vi], axis=2)        # [B, E, 80] f32
    # partition-major: e = c*128 + p  ->  [B, p, c, 80]
    xuv = np.ascontiguousarray(
        xuv.reshape(B, NCH, 128, 80).transpose(0, 2, 1, 3)
    )
    w16 = np.ascontiguousarray(
        np.concatenate([Ur, Ui], axis=2)
        .reshape(B, NCH, 128, 40)
        .transpose(0, 2, 1, 3)
        .astype(np.float16)
    )
    return [
        {
            "xuv": xuv[i * NB:(i + 1) * NB],
            "w16": w16[i * NB:(i + 1) * NB],
            "tkr": tkr16[i * NB:(i + 1) * NB],
            "tki": tki16[i * NB:(i + 1) * NB],
        }
        for i in range(NCORES)
    ]


def _run_device(nn, tkr, tki, trace=False):
    nc = _get_program()
    in_maps = _pack_inputs(nn, tkr, tki)
    return run_bass_kernel_spmd(nc, in_maps, list(range(NCORES)), trace=trace)


def _finalize(nn, results, batch_size):
    """Assemble (loss, obj1, obj2) from per-core device partials (float64)."""
    nn = np.asarray(nn)
    d = (nn[:, :K] + 1j * nn[:, NOUT:NOUT + K]).astype(np.complex128)
    Vr = nn[:, VLOC:NOUT].reshape(B, E, K).astype(np.float64)
    Vi = nn[:, NOUT + VLOC:2 * NOUT].reshape(B, E, K).astype(np.float64)
    V = Vr + 1j * Vi

    gram = np.concatenate(
        [r["gram"] for r in results], axis=0
    ).astype(np.float64)                                   # [B, 40, 80]
    yr = np.concatenate([r["yr"] for r in results], axis=0).astype(np.float64)
    yi = np.concatenate([r["yi"] for r in results], axis=0).astype(np.float64)
    den = float(sum(np.sum(r["den"], dtype=np.float64) for r in results))

    SU = gram[:, :, 0:40]
    SV = gram[:, :, 40:80]
    Srr = SU[:, 0:20, 0:20]
    Sri = SU[:, 0:20, 20:40]
    Sii = SU[:, 20:40, 20:40]
    Trr = SV[:, 0:20, 0:20]
    Tri = SV[:, 0:20, 20:40]
    Tii = SV[:, 20:40, 20:40]
    SriT = np.transpose(Sri, (0, 2, 1))
    TriT = np.transpose(Tri, (0, 2, 1))
    G_U = (Srr - Sii) + 1j * (Sri + SriT)
    G_V = (Trr - Tii) + 1j * (Tri + TriT)
    H_U = (Srr + Sii) + 1j * (Sri - SriT)
    H_V = (Trr + Tii) + 1j * (Tri - TriT)

    mask = np.triu(np.ones((K, K), dtype=bool), k=1)
    bsz = float(batch_size)
    obj1 = float(np.sum(np.abs(G_U)[:, mask]) / bsz)
    obj2 = float(np.sum(np.abs(G_V)[:, mask]) / bsz)

    prednorm = float(
        np.real(
            np.einsum("bk,bl,bkl,bkl->", d, np.conj(d), np.conj(H_U), np.conj(H_V))
        )
    )

    # cross = Re<conj(tk), pred>; Wc[b,k,f] = sum_e conj(tk[e,f]) U[e,k]
    Wc = (yr[:, 0:20, :] + yi[:, 20:40, :]) + 1j * (yr[:, 20:40, :] - yi[:, 0:20, :])
    zeta = np.einsum("bfk,bkf->bk", V, Wc)
    cross = float(np.real(np.einsum("bk,bk->", d, zeta)))

    num = den - 2.0 * cross + prednorm
    loss = num / den + PENALTY * (obj1 + obj2)
    return (
        np.float32(loss),
        np.float32(obj1),
        np.float32(obj2),
    )


def kernel(nnOutput, kern_real, kern_imag, batch_Size):
    nn = np.ascontiguousarray(np.asarray(nnOutput, dtype=np.float32))
    tkr = np.asarray(kern_real, dtype=np.float32)
    tki = np.asarray(kern_imag, dtype=np.float32)
    res = _run_device(nn, tkr, tki).results
    return _finalize(nn, res, int(batch_Size))



# revision 11
# speedup vs baseline: 1.9032x; 1.9032x over previous
"""Trainium2 Bass kernel for nn_CustomLoss_74826920231413.

Loss structure (B=32, E=1024, K=20):
    c  = complex(nnOutput[:, :NOUT], nnOutput[:, NOUT:])
    d  = c[:, :K];  U = c[:, K:VLOC].reshape(B,E,K);  V = c[:, VLOC:].reshape(B,E,K)
    obj1/obj2 = sum_{j<k} |U^T U| / B (no conj), same for V
    pred = U @ diag(d) @ V^T;  tk = complex(kern_real, kern_imag)
    loss = ||tk - pred||^2 / ||tk||^2 + 0.01*(obj1+obj2)

Device strategy (data-parallel over B, 4 batch rows per core, 8 cores):
    ||tk - pred||^2 = ||tk||^2 - 2*Re<conj(tk),pred> + ||pred||^2. The device
    streams tk once in fp8 (e4m3) and produces small per-batch outputs:
      * wc[b]   = W1^T tkr + W2^T tki with W1=[Ur|-Ui], W2=[Ui|Ur] (40x1024)
                  -> rows 0:20 = Re(Wc), rows 20:40 = Im(Wc) where
                  Wc[k,f] = sum_e conj(tk)[e,f] U[e,k]   (cross term)
      * gram[b] = [W1^T W1 | Vw^T Vw] with Vw=[Vr|Vi]  (40x80)
                  -> obj1, obj2, ||pred||^2
    den = ||tk||^2 is an exact fp64 dot on the host (the device already
    reads every tk byte for the cross term; recomputing squares on the
    DVE/ACT engines would throttle the stream).

    fp8 tolerance: the loss is a ratio of O(1e9) quantities; e4m3 rounding
    of tk/U/V perturbs (loss, obj1, obj2) at ~8e-4 relative (validated
    numerically) vs the 2e-2 gate. ml_dtypes.float8_e4m3 is bit-identical
    to TRN FP8_EXP4 for |x| <= 240.

    tk rides the sync HWDGE ring as 4x2MB host-prepacked partition-major
    DMAs; outputs ride the scalar HWDGE ring (2nd HW ring, no SWDGE
    drain). fp8 matmuls run in DoubleRow perf mode (2 contraction rows
    per cycle); dummy warmup matmuls keep the PE HAM un-throttled while
    the first DMA lands.
"""

import sys

for _p in ("/opt/trn_rl_repo", "/root/.axon_site/_ro/trn_rl_repo"):
    if _p not in sys.path:
        sys.path.append(_p)

import ml_dtypes
import numpy as np

import concourse.bacc as bacc
import concourse.mybir as mybir
import concourse.tile as tile
from concourse.bass_utils import run_bass_kernel_spmd

# Problem constants (hardcoded per harness contract)
E = 1024
K = 20
NOUT = K * (2 * E + 1)          # 40980
VLOC = K + K * E                # 20500
PENALTY = 0.01
B = 32
NCORES = 8
NB = B // NCORES                # batch rows per core
NCH = E // 128                  # 8 e-chunks of 128 partitions
F32 = mybir.dt.float32
F8 = mybir.dt.float8e4          # TRN FP8_EXP4 == ml_dtypes.float8_e4m3
NP_F8 = ml_dtypes.float8_e4m3

DOUBLE_ROW = True               # fp8 DoubleRow: 2 contraction rows/cycle

_PROGRAM_CACHE = {}


def _build_program():
    """Per-core SPMD Bass program. Same program on all 8 cores; each core
    receives its own 4-row slice of the inputs (host-packed layouts)."""
    nc = bacc.Bacc("TRN2", target_bir_lowering=False, debug=False)

    # fp8 kernels, partition-major: [b, p, ri, c, f], e = c*128+p.
    # 16KB contiguous per partition line per b -> few descriptors,
    # 2MB per dma_start.
    tk_d = nc.dram_tensor("tk8", [NB, 128, 2, NCH, E], F8, kind="ExternalInput").ap()
    # fp8 weights, zero-padded to 48 cols/chunk so every DoubleRow weight
    # slice offset and k-tile step is 16B-aligned (s3_lw_dual_fp8 ISA rule).
    # w1: cols 0:40 = [Ur|-Ui]; w2: cols 0:40 = [Ui|Ur]; v: cols 0:40 = [Vr|Vi]
    w1_d = nc.dram_tensor("w1", [128, NB, NCH, 48], F8, kind="ExternalInput").ap()
    w2_d = nc.dram_tensor("w2", [128, NB, NCH, 48], F8, kind="ExternalInput").ap()
    v_d = nc.dram_tensor("v8", [128, NB, NCH, 48], F8, kind="ExternalInput").ap()

    wc_d = nc.dram_tensor("wc", [NB, 40, E], F32, kind="ExternalOutput").ap()
    gram_d = nc.dram_tensor("gram", [NB, 40, 80], F32, kind="ExternalOutput").ap()

    DR = mybir.MatmulPerfMode.DoubleRow if DOUBLE_ROW else None

    with tile.TileContext(nc) as tc:
        with (
            tc.tile_pool(name="tk", bufs=1) as tkpool,
            tc.tile_pool(name="wv", bufs=1) as wvpool,
            tc.tile_pool(name="evac", bufs=2) as evacpool,
            tc.tile_pool(name="pswc", bufs=2, space="PSUM") as pswc_pool,
            tc.tile_pool(name="psg", bufs=2, space="PSUM") as psg_pool,
            tc.tile_pool(name="pswu", bufs=1, space="PSUM") as pswu_pool,
        ):
            # ---- input DMAs up front, in stream order (sync ring is FIFO)
            w1_sb = wvpool.tile([128, NB, NCH, 48], F8, name="w1_sb")
            nc.sync.dma_start(w1_sb[:], w1_d[:])
            w2_sb = wvpool.tile([128, NB, NCH, 48], F8, name="w2_sb")
            nc.sync.dma_start(w2_sb[:], w2_d[:])
            v_sb = wvpool.tile([128, NB, NCH, 48], F8, name="v_sb")
            nc.sync.dma_start(v_sb[:], v_d[:])
            tk_sb = []
            for b in range(NB):
                t = tkpool.tile([128, 2, NCH, E], F8, name=f"tk_sb{b}")
                nc.sync.dma_start(t[:], tk_d[b])
                tk_sb.append(t)

            # ---- PE warmup: keep HAM at 8/8 while the first DMA lands.
            # Garbage-free: warm tile is memset, psum never read.
            warm_sb = wvpool.tile([128, 512], F8, name="warm_sb")
            nc.vector.memset(warm_sb[:], 1.0)
            ps_warm = pswu_pool.tile([64, 512], F32, name="ps_warm")
            for i in range(24):
                nc.tensor.matmul(
                    ps_warm[:], warm_sb[:, 0:64], warm_sb[:], start=True, stop=True
                )

            for b in range(NB):
                ps_wc = pswc_pool.tile([40, E], F32, name="ps_wc")
                ps_g = psg_pool.tile([40, 80], F32, name="ps_g")

                if DOUBLE_ROW:
                    # wc: accumulate W1^T tkr then W2^T tki, 2 chunks/matmul
                    for ri, wsb in ((0, w1_sb), (1, w2_sb)):
                        for cp in range(NCH // 2):
                            cs = slice(2 * cp, 2 * cp + 2)
                            w = wsb[:, b, cs, 0:40]
                            for h in range(2):
                                fs = slice(h * 512, (h + 1) * 512)
                                nc.tensor.matmul(
                                    ps_wc[:, fs],
                                    w,
                                    tk_sb[b][:, ri, cs, fs],
                                    start=(ri == 0 and cp == 0),
                                    stop=(ri == 1 and cp == NCH // 2 - 1),
                                    perf_mode=DR,
                                )
                    # grams: U from W1 (sign-fixed on host), V from [Vr|Vi].
                    # NOTE: groups must NOT interleave within the shared PSUM
                    # bank — start=True marks the whole 2KB bank pending-zero,
                    # wiping any other in-flight group's partials there.
                    for cp in range(NCH // 2):
                        cs = slice(2 * cp, 2 * cp + 2)
                        w1 = w1_sb[:, b, cs, 0:40]
                        nc.tensor.matmul(
                            ps_g[:, 0:40],
                            w1,
                            w1,
                            start=(cp == 0),
                            stop=(cp == NCH // 2 - 1),
                            perf_mode=DR,
                        )
                    for cp in range(NCH // 2):
                        cs = slice(2 * cp, 2 * cp + 2)
                        vv = v_sb[:, b, cs, 0:40]
                        nc.tensor.matmul(
                            ps_g[:, 40:80],
                            vv,
                            vv,
                            start=(cp == 0),
                            stop=(cp == NCH // 2 - 1),
                            perf_mode=DR,
                        )
                else:
                    for ri, wsb in ((0, w1_sb), (1, w2_sb)):
                        for c in range(NCH):
                            w = wsb[:, b, c, 0:40]
                            for h in range(2):
                                fs = slice(h * 512, (h + 1) * 512)
                                nc.tensor.matmul(
                                    ps_wc[:, fs],
                                    w,
                                    tk_sb[b][:, ri, c, fs],
                                    start=(ri == 0 and c == 0),
                                    stop=(ri == 1 and c == NCH - 1),
                                )
                    for c in range(NCH):
                        w1 = w1_sb[:, b, c, 0:40]
                        nc.tensor.matmul(
                            ps_g[:, 0:40], w1, w1, start=(c == 0), stop=(c == NCH - 1)
                        )
                    for c in range(NCH):
                        vv = v_sb[:, b, c, 0:40]
                        nc.tensor.matmul(
                            ps_g[:, 40:80], vv, vv, start=(c == 0), stop=(c == NCH - 1)
                        )

                # ---- evacuate PSUM (DVE + ACT in parallel) and store on the
                # scalar HWDGE ring (distinct from the input sync ring)
                wc_sb = evacpool.tile([40, E], F32, name="wc_sb")
                nc.vector.tensor_copy(wc_sb[:, 0:512], ps_wc[:, 0:512])
                nc.scalar.copy(wc_sb[:, 512:1024], ps_wc[:, 512:1024])
                nc.scalar.dma_start(wc_d[b], wc_sb[:])
                g_sb = evacpool.tile([40, 80], F32, name="g_sb")
                nc.vector.tensor_copy(g_sb[:], ps_g[:])
                nc.scalar.dma_start(gram_d[b], g_sb[:])

    nc.compile()
    return nc


def _get_program():
    if "nc" not in _PROGRAM_CACHE:
        _PROGRAM_CACHE["nc"] = _build_program()
    return _PROGRAM_CACHE["nc"]


def _pack_inputs(nn, tkr, tki):
    """Host-side packing: per-core input dicts with device-friendly fp8
    layouts."""
    # partition-major fp8: [B, E, E] -> [B, p, c, f] with e = c*128 + p
    def pack_tk(a):
        return np.ascontiguousarray(
            a.astype(NP_F8).reshape(B, NCH, 128, E).transpose(0, 2, 1, 3)
        )

    tk8 = np.stack([pack_tk(tkr), pack_tk(tki)], axis=2)  # [B, 128, 2, NCH, E]

    # [B, E, K] slices of nn
    Ur = nn[:, K:VLOC].reshape(B, E, K)
    Ui = nn[:, NOUT + K:NOUT + VLOC].reshape(B, E, K)
    Vr = nn[:, VLOC:NOUT].reshape(B, E, K)
    Vi = nn[:, NOUT + VLOC:2 * NOUT].reshape(B, E, K)
    pad = np.zeros((B, E, 8), np.float32)

    def pack_w(a, b_):
        # [p, b, c, 48]: cols 0:40 = [a|b_], 40:48 zero pad (16B alignment)
        w = np.concatenate([a, b_, pad], axis=2).astype(NP_F8)
        return np.ascontiguousarray(
            w.reshape(B, NCH, 128, 48).transpose(2, 0, 1, 3)
        )

    w1 = pack_w(Ur, -Ui)
    w2 = pack_w(Ui, Ur)
    v8 = pack_w(Vr, Vi)
    return [
        {
            "tk8": tk8[i * NB:(i + 1) * NB],
            "w1": w1[:, i * NB:(i + 1) * NB],
            "w2": w2[:, i * NB:(i + 1) * NB],
            "v8": v8[:, i * NB:(i + 1) * NB],
        }
        for i in range(NCORES)
    ]


def _run_device(nn, tkr, tki, trace=False):
    nc = _get_program()
    in_maps = _pack_inputs(nn, tkr, tki)
    return run_bass_kernel_spmd(nc, in_maps, list(range(NCORES)), trace=trace)


def _den_f64(a):
    f = a.ravel().astype(np.float64)
    return float(np.dot(f, f))


def _finalize(nn, tkr, tki, results, batch_size):
    """Assemble (loss, obj1, obj2) from per-core device partials (float64)."""
    nn = np.asarray(nn)
    d = (nn[:, :K] + 1j * nn[:, NOUT:NOUT + K]).astype(np.complex128)
    Vr = nn[:, VLOC:NOUT].reshape(B, E, K).astype(np.float64)
    Vi = nn[:, NOUT + VLOC:2 * NOUT].reshape(B, E, K).astype(np.float64)
    V = Vr + 1j * Vi

    wc = np.concatenate([r["wc"] for r in results], axis=0).astype(np.float64)
    gram = np.concatenate(
        [r["gram"] for r in results], axis=0
    ).astype(np.float64)                                   # [B, 40, 80]

    den = _den_f64(tkr) + _den_f64(tki)

    # cross = Re<conj(tk), pred>; Wc[b,k,f] = sum_e conj(tk[e,f]) U[e,k]
    Wc = wc[:, 0:20, :] + 1j * wc[:, 20:40, :]
    zeta = np.einsum("bfk,bkf->bk", V, Wc)
    cross = float(np.real(np.einsum("bk,bk->", d, zeta)))

    # gram[:, :, 0:40] = W1^T W1 with W1 = [Ur|-Ui]:
    #   [[Ur^T Ur, -Ur^T Ui], [-Ui^T Ur, Ui^T Ui]] -> flip sign of Sri
    gU = gram[:, :, 0:40]
    gV = gram[:, :, 40:80]
    Srr = gU[:, 0:20, 0:20]
    Sri = -gU[:, 0:20, 20:40]
    Sii = gU[:, 20:40, 20:40]
    Trr = gV[:, 0:20, 0:20]
    Tri = gV[:, 0:20, 20:40]
    Tii = gV[:, 20:40, 20:40]
    SriT = np.transpose(Sri, (0, 2, 1))
    TriT = np.transpose(Tri, (0, 2, 1))
    G_U = (Srr - Sii) + 1j * (Sri + SriT)
    G_V = (Trr - Tii) + 1j * (Tri + TriT)
    H_U = (Srr + Sii) + 1j * (Sri - SriT)
    H_V = (Trr + Tii) + 1j * (Tri - TriT)

    mask = np.triu(np.ones((K, K), dtype=bool), k=1)
    bsz = float(batch_size)
    obj1 = float(np.sum(np.abs(G_U)[:, mask]) / bsz)
    obj2 = float(np.sum(np.abs(G_V)[:, mask]) / bsz)

    prednorm = float(
        np.real(
            np.einsum("bk,bl,bkl,bkl->", d, np.conj(d), np.conj(H_U), np.conj(H_V))
        )
    )

    num = den - 2.0 * cross + prednorm
    loss = num / den + PENALTY * (obj1 + obj2)
    return (
        np.float32(loss),
        np.float32(obj1),
        np.float32(obj2),
    )


def kernel(nnOutput, kern_real, kern_imag, batch_Size):
    nn = np.ascontiguousarray(np.asarray(nnOutput, dtype=np.float32))
    tkr = np.asarray(kern_real, dtype=np.float32)
    tki = np.asarray(kern_imag, dtype=np.float32)
    res = _run_device(nn, tkr, tki).results
    return _finalize(nn, tkr, tki, res, int(batch_Size))


# revision 13
# speedup vs baseline: 1.9568x; 1.0282x over previous
"""Trainium2 Bass kernel for nn_CustomLoss_74826920231413.

Loss structure (B=32, E=1024, K=20):
    c  = complex(nnOutput[:, :NOUT], nnOutput[:, NOUT:])
    d  = c[:, :K];  U = c[:, K:VLOC].reshape(B,E,K);  V = c[:, VLOC:].reshape(B,E,K)
    obj1/obj2 = sum_{j<k} |U^T U| / B (no conj), same for V
    pred = U @ diag(d) @ V^T;  tk = complex(kern_real, kern_imag)
    loss = ||tk - pred||^2 / ||tk||^2 + 0.01*(obj1+obj2)

Device strategy (data-parallel over B, 4 batch rows per core, 8 cores):
    ||tk - pred||^2 = ||tk||^2 - 2*Re<conj(tk),pred> + ||pred||^2. The device
    streams tk once in fp8 (e4m3) and produces small per-batch outputs:
      * wc[b]   = W1^T tkr + W2^T tki with W1=[Ur|-Ui], W2=[Ui|Ur] (40x1024)
                  -> rows 0:20 = Re(Wc), rows 20:40 = Im(Wc) where
                  Wc[k,f] = sum_e conj(tk)[e,f] U[e,k]   (cross term)
      * gram[b] = [W1^T W1 | Vw^T Vw] with Vw=[Vr|Vi]  (40x80)
                  -> obj1, obj2, ||pred||^2
    den = ||tk||^2 is an exact fp64 dot on the host (the device already
    reads every tk byte for the cross term; recomputing squares on the
    DVE/ACT engines would throttle the stream).

    fp8 tolerance: the loss is a ratio of O(1e9) quantities; e4m3 rounding
    of tk/U/V perturbs (loss, obj1, obj2) at ~8e-4 relative (validated
    numerically) vs the 2e-2 gate. ml_dtypes.float8_e4m3 is bit-identical
    to TRN FP8_EXP4 for |x| <= 240.

    tk rides the sync HWDGE ring as 4x2MB host-prepacked partition-major
    DMAs; outputs ride the scalar HWDGE ring (2nd HW ring, no SWDGE
    drain). fp8 matmuls run in DoubleRow perf mode (2 contraction rows
    per cycle); dummy warmup matmuls keep the PE HAM un-throttled while
    the first DMA lands.
"""

import sys

for _p in ("/opt/trn_rl_repo", "/root/.axon_site/_ro/trn_rl_repo"):
    if _p not in sys.path:
        sys.path.append(_p)

import ml_dtypes
import numpy as np

import concourse.bacc as bacc
import concourse.mybir as mybir
import concourse.tile as tile
from concourse.bass_utils import run_bass_kernel_spmd

# Problem constants (hardcoded per harness contract)
E = 1024
K = 20
NOUT = K * (2 * E + 1)          # 40980
VLOC = K + K * E                # 20500
PENALTY = 0.01
B = 32
NCORES = 8
NB = B // NCORES                # batch rows per core
NCH = E // 128                  # 8 e-chunks of 128 partitions
F32 = mybir.dt.float32
F8 = mybir.dt.float8e4          # TRN FP8_EXP4 == ml_dtypes.float8_e4m3
NP_F8 = ml_dtypes.float8_e4m3

DOUBLE_ROW = True               # fp8 DoubleRow: 2 contraction rows/cycle

_PROGRAM_CACHE = {}


def _build_program():
    """Per-core SPMD Bass program. Same program on all 8 cores; each core
    receives its own 4-row slice of the inputs (host-packed layouts)."""
    nc = bacc.Bacc("TRN2", target_bir_lowering=False, debug=False)

    # fp8 kernels, partition-major: [b, p, ri, c, f], e = c*128+p.
    # 16KB contiguous per partition line per b -> few descriptors,
    # 2MB per dma_start.
    tk_d = nc.dram_tensor("tk8", [NB, 128, 2, NCH, E], F8, kind="ExternalInput").ap()
    # fp8 weights, zero-padded to 48 cols/chunk so every DoubleRow weight
    # slice offset and k-tile step is 16B-aligned (s3_lw_dual_fp8 ISA rule).
    # w1: cols 0:40 = [Ur|-Ui]; w2: cols 0:40 = [Ui|Ur]; v: cols 0:40 = [Vr|Vi]
    w1_d = nc.dram_tensor("w1", [128, NB, NCH, 48], F8, kind="ExternalInput").ap()
    w2_d = nc.dram_tensor("w2", [128, NB, NCH, 48], F8, kind="ExternalInput").ap()
    v_d = nc.dram_tensor("v8", [128, NB, NCH, 48], F8, kind="ExternalInput").ap()

    wc_d = nc.dram_tensor("wc", [NB, 40, E], F32, kind="ExternalOutput").ap()
    gram_d = nc.dram_tensor("gram", [NB, 40, 80], F32, kind="ExternalOutput").ap()

    DR = mybir.MatmulPerfMode.DoubleRow if DOUBLE_ROW else None

    # DoubleRow processes chunk PAIRS (contraction 256); the fallback single
    # chunks. Emit helpers keep the two paths structurally identical.
    CG = 2 if DOUBLE_ROW else 1          # chunks per matmul
    NMM = NCH // CG                      # matmuls per (ri, half)

    with tile.TileContext(nc) as tc:
        with (
            tc.tile_pool(name="tk", bufs=1) as tkpool,
            tc.tile_pool(name="wv", bufs=1) as wvpool,
            tc.tile_pool(name="evac", bufs=4) as evacpool,
            tc.tile_pool(name="gevac", bufs=4) as gevacpool,
            tc.tile_pool(name="pswc", bufs=2, space="PSUM") as pswc_pool,
            tc.tile_pool(name="psg", bufs=2, space="PSUM") as psg_pool,
            tc.tile_pool(name="pswu", bufs=1, space="PSUM") as pswu_pool,
        ):
            # ---- input DMAs up front. Two HWDGE rings stream in parallel:
            # sync ring: w1, w2, tkr[0..3]; scalar ring: v8, tki[0..3].
            # Outputs ride the sync ring behind the loads (FIFO; loads drain
            # first, stores are tiny and late anyway).
            w1_sb = wvpool.tile([128, NB, NCH, 48], F8, name="w1_sb")
            nc.sync.dma_start(w1_sb[:], w1_d[:])
            w2_sb = wvpool.tile([128, NB, NCH, 48], F8, name="w2_sb")
            nc.sync.dma_start(w2_sb[:], w2_d[:])
            v_sb = wvpool.tile([128, NB, NCH, 48], F8, name="v_sb")
            nc.scalar.dma_start(v_sb[:], v_d[:])
            tk_sb = []
            for b in range(NB):
                t = tkpool.tile([128, 2, NCH, E], F8, name=f"tk_sb{b}")
                nc.sync.dma_start(t[:, 0], tk_d[b, :, 0])
                nc.scalar.dma_start(t[:, 1], tk_d[b, :, 1])
                tk_sb.append(t)

            # ---- PE warmup: keep HAM at 8/8 while the first DMAs land.
            warm_sb = wvpool.tile([128, 512], F8, name="warm_sb")
            nc.vector.memset(warm_sb[:], 1.0)
            ps_warm = pswu_pool.tile([64, 512], F32, name="ps_warm")
            for i in range(14):
                nc.tensor.matmul(
                    ps_warm[:], warm_sb[:, 0:64], warm_sb[:], start=True, stop=True
                )

            # ---- grams for all batches up front (only need w1/v8, which
            # arrive first): U from W1 (sign-fixed on host), V from [Vr|Vi].
            # NOTE: the two groups share a PSUM bank and must NOT interleave —
            # start=True marks the whole 2KB bank pending-zero, wiping any
            # other in-flight group's partials there.
            g_sbs = []
            for b in range(NB):
                ps_g = psg_pool.tile([40, 80], F32, name="ps_g")
                for i in range(NMM):
                    cs = slice(CG * i, CG * (i + 1))
                    w1 = w1_sb[:, b, cs, 0:40]
                    nc.tensor.matmul(
                        ps_g[:, 0:40], w1, w1,
                        start=(i == 0), stop=(i == NMM - 1), perf_mode=DR,
                    )
                for i in range(NMM):
                    cs = slice(CG * i, CG * (i + 1))
                    vv = v_sb[:, b, cs, 0:40]
                    nc.tensor.matmul(
                        ps_g[:, 40:80], vv, vv,
                        start=(i == 0), stop=(i == NMM - 1), perf_mode=DR,
                    )
                g_sb = gevacpool.tile([40, 80], F32, name="g_sb")
                nc.vector.tensor_copy(g_sb[:], ps_g[:])
                nc.sync.dma_start(gram_d[b], g_sb[:])
                g_sbs.append(g_sb)

            # ---- wc accumulation. tki (scalar ring) lands ~1.5us before
            # tkr (sync ring carries more), so consume tki first. Each 512-col
            # half is its own PSUM bank/group; finish h0's group before h1's
            # last matmuls so its evac overlaps the remaining matmuls.
            for b in range(NB):
                ps_wc = pswc_pool.tile([40, E], F32, name="ps_wc")
                # in-order engine: consume tki fully first (it lands earlier),
                # and order tkr h0-then-h1 so h0's group closes 4 matmuls
                # before h1's, letting its evac overlap h1's tail.
                for oi, (ri, wsb) in enumerate(((1, w2_sb), (0, w1_sb))):
                    for h in range(2):
                        fs = slice(h * 512, (h + 1) * 512)
                        for i in range(NMM):
                            cs = slice(CG * i, CG * (i + 1))
                            nc.tensor.matmul(
                                ps_wc[:, fs],
                                wsb[:, b, cs, 0:40],
                                tk_sb[b][:, ri, cs, fs],
                                start=(oi == 0 and i == 0),
                                stop=(oi == 1 and i == NMM - 1),
                                perf_mode=DR,
                            )
                wc_sb = evacpool.tile([40, E], F32, name="wc_sb")
                nc.vector.tensor_copy(wc_sb[:, 0:512], ps_wc[:, 0:512])
                nc.scalar.copy(wc_sb[:, 512:1024], ps_wc[:, 512:1024])
                nc.sync.dma_start(wc_d[b], wc_sb[:])

    nc.compile()
    return nc


def _get_program():
    if "nc" not in _PROGRAM_CACHE:
        _PROGRAM_CACHE["nc"] = _build_program()
    return _PROGRAM_CACHE["nc"]


def _pack_inputs(nn, tkr, tki):
    """Host-side packing: per-core input dicts with device-friendly fp8
    layouts."""
    # partition-major fp8: [B, E, E] -> [B, p, c, f] with e = c*128 + p
    def pack_tk(a):
        return np.ascontiguousarray(
            a.astype(NP_F8).reshape(B, NCH, 128, E).transpose(0, 2, 1, 3)
        )

    tk8 = np.stack([pack_tk(tkr), pack_tk(tki)], axis=2)  # [B, 128, 2, NCH, E]

    # [B, E, K] slices of nn
    Ur = nn[:, K:VLOC].reshape(B, E, K)
    Ui = nn[:, NOUT + K:NOUT + VLOC].reshape(B, E, K)
    Vr = nn[:, VLOC:NOUT].reshape(B, E, K)
    Vi = nn[:, NOUT + VLOC:2 * NOUT].reshape(B, E, K)
    pad = np.zeros((B, E, 8), np.float32)

    def pack_w(a, b_):
        # [p, b, c, 48]: cols 0:40 = [a|b_], 40:48 zero pad (16B alignment)
        w = np.concatenate([a, b_, pad], axis=2).astype(NP_F8)
        return np.ascontiguousarray(
            w.reshape(B, NCH, 128, 48).transpose(2, 0, 1, 3)
        )

    w1 = pack_w(Ur, -Ui)
    w2 = pack_w(Ui, Ur)
    v8 = pack_w(Vr, Vi)
    return [
        {
            "tk8": tk8[i * NB:(i + 1) * NB],
            "w1": w1[:, i * NB:(i + 1) * NB],
            "w2": w2[:, i * NB:(i + 1) * NB],
            "v8": v8[:, i * NB:(i + 1) * NB],
        }
        for i in range(NCORES)
    ]


def _run_device(nn, tkr, tki, trace=False):
    nc = _get_program()
    in_maps = _pack_inputs(nn, tkr, tki)
    return run_bass_kernel_spmd(nc, in_maps, list(range(NCORES)), trace=trace)


def _den_f64(a):
    f = a.ravel().astype(np.float64)
    return float(np.dot(f, f))


def _finalize(nn, tkr, tki, results, batch_size):
    """Assemble (loss, obj1, obj2) from per-core device partials (float64)."""
    nn = np.asarray(nn)
    d = (nn[:, :K] + 1j * nn[:, NOUT:NOUT + K]).astype(np.complex128)
    Vr = nn[:, VLOC:NOUT].reshape(B, E, K).astype(np.float64)
    Vi = nn[:, NOUT + VLOC:2 * NOUT].reshape(B, E, K).astype(np.float64)
    V = Vr + 1j * Vi

    wc = np.concatenate([r["wc"] for r in results], axis=0).astype(np.float64)
    gram = np.concatenate(
        [r["gram"] for r in results], axis=0
    ).astype(np.float64)                                   # [B, 40, 80]

    den = _den_f64(tkr) + _den_f64(tki)

    # cross = Re<conj(tk), pred>; Wc[b,k,f] = sum_e conj(tk[e,f]) U[e,k]
    Wc = wc[:, 0:20, :] + 1j * wc[:, 20:40, :]
    zeta = np.einsum("bfk,bkf->bk", V, Wc)
    cross = float(np.real(np.einsum("bk,bk->", d, zeta)))

    # gram[:, :, 0:40] = W1^T W1 with W1 = [Ur|-Ui]:
    #   [[Ur^T Ur, -Ur^T Ui], [-Ui^T Ur, Ui^T Ui]] -> flip sign of Sri
    gU = gram[:, :, 0:40]
    gV = gram[:, :, 40:80]
    Srr = gU[:, 0:20, 0:20]
    Sri = -gU[:, 0:20, 20:40]
    Sii = gU[:, 20:40, 20:40]
    Trr = gV[:, 0:20, 0:20]
    Tri = gV[:, 0:20, 20:40]
    Tii = gV[:, 20:40, 20:40]
    SriT = np.transpose(Sri, (0, 2, 1))
    TriT = np.transpose(Tri, (0, 2, 1))
    G_U = (Srr - Sii) + 1j * (Sri + SriT)
    G_V = (Trr - Tii) + 1j * (Tri + TriT)
    H_U = (Srr + Sii) + 1j * (Sri - SriT)
    H_V = (Trr + Tii) + 1j * (Tri - TriT)

    mask = np.triu(np.ones((K, K), dtype=bool), k=1)
    bsz = float(batch_size)
    obj1 = float(np.sum(np.abs(G_U)[:, mask]) / bsz)
    obj2 = float(np.sum(np.abs(G_V)[:, mask]) / bsz)

    prednorm = float(
        np.real(
            np.einsum("bk,bl,bkl,bkl->", d, np.conj(d), np.conj(H_U), np.conj(H_V))
        )
    )

    num = den - 2.0 * cross + prednorm
    loss = num / den + PENALTY * (obj1 + obj2)
    return (
        np.float32(loss),
        np.float32(obj1),
        np.float32(obj2),
    )


def kernel(nnOutput, kern_real, kern_imag, batch_Size):
    nn = np.ascontiguousarray(np.asarray(nnOutput, dtype=np.float32))
    tkr = np.asarray(kern_real, dtype=np.float32)
    tki = np.asarray(kern_imag, dtype=np.float32)
    res = _run_device(nn, tkr, tki).results
    return _finalize(nn, tkr, tki, res, int(batch_Size))
